# revision 1
# baseline (speedup 1.0000x reference)
"""DigitCaps kernel for 8 Trainium2 NeuronCores.

Math (per batch b):
    U_hat[b,d,n,j] = sum_i W[d,n,j,i] * u[b,n,i]
    A_sum[b,d,m]   = sum_n U_hat[b,d,n,:] . U_hat[b,d,m,:] / sqrt(dp)
                   = s[b,d,:] . U_hat[b,d,m,:] / sqrt(dp),  s = sum_n U_hat
    C              = softmax_d(A_sum)
    S[b,d,j]       = sum_m (B_prior[d,m] + C[b,d,m]) * U_hat[b,d,m,j]
    out            = squash(S)

The huge [B,D,N,N] similarity matrix collapses because it is immediately
summed over n - only the n-sum s of U_hat is needed.

Sharding: data-parallel over batch, 2 batches per core, W/B_prior replicated.
Inputs are pre-arranged on the host into per-tile layouts so every device DMA
reads fully contiguous memory.

Per-core layout: n-tiles of 128 on partitions.
    W_sb[nt]  : [n=128, (d,j,i)=1280]   (host-arranged, contiguous 5KB rows)
    U2[nt]    : [n=128, (b,d,j)=320]    multiply-accumulate chain over i
                (DVE TensorScalarPtr; 4 half-chains on GpSimd as mult+tree)
    s         : ones.T @ U2 fp32        (PE partition-reduce, per-batch-half
                                         PSUM groups; all rows equal s)
    then per n-tile (pipelined): A_sum (GpSimd mult + DVE reduce) -> exp
    (ACT, scale=1/sqrt(dp), table preloaded) -> softmax_d norm (DVE) ->
    +B_prior -> S matmul (PE, bf16, PSUM accum);
    diagonal extract via iota mask; squash with Newton sqrt on DVE (keeps
    the Exp ACT table resident - no table reloads in the tail).
"""

import math
import numpy as np

import concourse.bacc as bacc
import concourse.bass as bass
import concourse.tile as tile
from concourse import mybir
from concourse.bass_utils import run_bass_kernel_spmd

F32 = mybir.dt.float32
I32 = mybir.dt.int32
AX = mybir.AxisListType
OP = mybir.AluOpType
ACTF = mybir.ActivationFunctionType

B, N, DP = 16, 1152, 8
D, DD = 10, 16
NCORES = 8
BPC = B // NCORES            # 2 batches per core
NT = N // 128                # 9 n-tiles
FW = D * DD * DP             # 1280 W free size
FD = D * DD                  # 160 per-batch U2 free size
FU = BPC * FD                # 320 U2 free size
NBD = BPC * D                # 20 (b,d) pairs
EPS = 1e-7
INV_SQRT_DP = 1.0 / math.sqrt(DP)


def _build_kernel(tc: "tile.TileContext", out_ap, WUB):
    nc = tc.nc
    with (
        tc.tile_pool(name="wpool", bufs=NT) as wpool,
        tc.tile_pool(name="tapool", bufs=6) as tapool,
        tc.tile_pool(name="ppool", bufs=4) as ppool,
        tc.tile_pool(name="smpool", bufs=2) as smpool,
        tc.tile_pool(name="persist", bufs=1) as persist,
        tc.tile_pool(name="psum_s", bufs=1, space="PSUM") as psum_s,
        tc.tile_pool(name="psum_S2", bufs=1, space="PSUM") as psum_S2,
    ):
        BF16 = mybir.dt.bfloat16
        ones_t = persist.tile([128, 128], F32, tag="ones")
        nc.vector.memset(ones_t[:], 1.0)

        u2_all = persist.tile([128, NT * FU], F32, tag="u2all")
        u2bf_all = persist.tile([128, NT * FU], BF16, tag="u2bfall")
        cbbf_all = persist.tile([128, NT * NBD], BF16, tag="cbbfall")
        cb_all = persist.tile([128, NT * NBD], F32, tag="cball")
        e_all = persist.tile([128, NT * NBD], F32, tag="eall")
        z_all = persist.tile([128, NT * BPC], F32, tag="zall")
        zr_all = persist.tile([128, NT * BPC], F32, tag="zrall")

        s_ps_0 = psum_s.tile([128, FD], F32, tag="sps0")
        s_ps_1 = psum_s.tile([128, FD], F32, tag="sps1")
        s_ps_b = [s_ps_0, s_ps_1]

        # preload the Exp ACT table while ACT is idle (hides the ~1.3us
        # table load that would otherwise land in the phase-2 critical path)
        warm_t = persist.tile([1, 1], F32, tag="warm")
        nc.vector.memset(warm_t[:], 0.0)
        nc.scalar.activation(warm_t[:], warm_t[:], ACTF.Exp)

        # ---- phase 1: load; U2 votes via i-chain; running s on PE ----
        ACT_TILES = ()  # these tiles route products via ACT + GpSimd
        # (nt, b) half-chains routed to GpSimd (mult + tree, all Pool-legal)
        POOL_HALVES = {(1, 1), (3, 1), (5, 1), (7, 1)}
        w_tiles = []
        for nt in range(NT):
            w_t = wpool.tile([128, FW + BPC * DP + D], F32, tag="w")
            w_tiles.append(w_t)
            nc.sync.dma_start(w_t[:], WUB[nt])
            u_t = w_t[:, FW: FW + BPC * DP]

            # U2[n,(b,d,j)] += W[n,(d,j,i)] * u[n,(b,i)]  accumulated over i.
            # TensorScalarPtr is DVE-only on trn2 (walrus rejects it on Pool),
            # so offload tiles via ACT products + GpSimd tree-reduce instead.
            w_3 = w_t[:, :FW].rearrange("p (dj i) -> p dj i", dj=FD, i=DP)
            if nt in ACT_TILES:
                # products P[n,(b,dj,i)] on ACT (Copy with per-partition
                # scale), then i-tree-reduce on GpSimd
                pp = ppool.tile([128, BPC * FW], F32, tag="pp")
                pp_v = pp[:].rearrange(
                    "p (b dj i) -> p b dj i", b=BPC, dj=FD, i=DP
                )
                for b in range(BPC):
                    for i in range(DP):
                        nc.scalar.activation(
                            pp_v[:, b, :, i],
                            w_3[:, :, i],
                            ACTF.Copy,
                            scale=u_t[:, b * DP + i: b * DP + i + 1],
                        )
                t1 = ppool.tile([128, BPC * FD * 4], F32, tag="t1")
                t1_v = t1[:].rearrange("p (g i) -> p g i", g=BPC * FD, i=4)
                pp_g = pp[:].rearrange("p (g i) -> p g i", g=BPC * FD, i=DP)
                nc.gpsimd.tensor_tensor(
                    t1_v, pp_g[:, :, 0:4], pp_g[:, :, 4:8], OP.add
                )
                t2 = ppool.tile([128, BPC * FD * 2], F32, tag="t2")
                t2_v = t2[:].rearrange("p (g i) -> p g i", g=BPC * FD, i=2)
                nc.gpsimd.tensor_tensor(
                    t2_v, t1_v[:, :, 0:2], t1_v[:, :, 2:4], OP.add
                )
                nc.gpsimd.tensor_tensor(
                    u2_all[:, nt * FU:(nt + 1) * FU].rearrange(
                        "p (g i) -> p g i", g=BPC * FD, i=1
                    ),
                    t2_v[:, :, 0:1],
                    t2_v[:, :, 1:2],
                    OP.add,
                )
            else:
                for b in range(BPC):
                    u2_sl = u2_all[:, nt * FU + b * FD: nt * FU + (b + 1) * FD]
                    if (nt, b) in POOL_HALVES:
                        # GpSimd route: one big mult + 3 tree-adds over i
                        pp = ppool.tile([128, FW], F32, tag="pp")
                        pp_v = pp[:].rearrange("p (g i) -> p g i", g=FD, i=DP)
                        u_bc = (
                            u_t[:, b * DP:(b + 1) * DP]
                            .unsqueeze(1)
                            .broadcast_to([128, FD, DP])
                        )
                        nc.gpsimd.tensor_tensor(pp_v, w_3, u_bc, OP.mult)
                        t1 = ppool.tile([128, FD * 4], F32, tag="t1")
                        t1_v = t1[:].rearrange("p (g i) -> p g i", g=FD, i=4)
                        nc.gpsimd.tensor_tensor(
                            t1_v, pp_v[:, :, 0:4], pp_v[:, :, 4:8], OP.add
                        )
                        t2 = ppool.tile([128, FD * 2], F32, tag="t2")
                        t2_v = t2[:].rearrange("p (g i) -> p g i", g=FD, i=2)
                        nc.gpsimd.tensor_tensor(
                            t2_v, t1_v[:, :, 0:2], t1_v[:, :, 2:4], OP.add
                        )
                        nc.gpsimd.tensor_tensor(
                            u2_sl.rearrange("p (g i) -> p g i", g=FD, i=1),
                            t2_v[:, :, 0:1],
                            t2_v[:, :, 1:2],
                            OP.add,
                        )
                        nc.tensor.matmul(
                            s_ps_b[b][:],
                            ones_t[:],
                            u2_sl,
                            start=(nt == 0),
                            stop=(nt == NT - 1),
                        )
                        continue
                    # first product on ACT (Copy with per-partition scale)
                    # frees two DVE ops per tile
                    nc.scalar.activation(
                        u2_sl,
                        w_3[:, :, 0],
                        ACTF.Copy,
                        scale=u_t[:, b * DP: b * DP + 1],
                    )
                    for i in range(1, DP):
                        nc.vector.scalar_tensor_tensor(
                            u2_sl,
                            w_3[:, :, i],
                            u_t[:, b * DP + i: b * DP + i + 1],
                            u2_sl,
                            OP.mult,
                            OP.add,
                        )
                    # s accumulation for this half-chain (fp32, PE idle;
                    # column-split groups give finer start dependencies)
                    nc.tensor.matmul(
                        s_ps_b[b][:],
                        ones_t[:],
                        u2_sl,
                        start=(nt == 0),
                        stop=(nt == NT - 1),
                    )



        # ---- phase 2 (pipelined per n-tile): A_sum -> softmax_d -> +B_prior
        #      -> S matmul ----
        # s copy to SBUF so GpSimd (no PSUM access) can read it (DVE: the
        # chain engine is free here and ACT's queue is backlogged)
        s_sb = persist.tile([128, FU], F32, tag="ssb")
        for b in range(BPC):
            nc.vector.tensor_copy(s_sb[:, b * FD:(b + 1) * FD], s_ps_b[b][:])

        # bf16 shadow of U2 for the S2 matmuls - cast lazily here, where ACT
        # is otherwise idle and off the phase-1 -> phase-2 critical path
        for nt in range(NT):
            nc.scalar.copy(
                u2bf_all[:, nt * FU:(nt + 1) * FU],
                u2_all[:, nt * FU:(nt + 1) * FU],
            )

        S2_ps = psum_S2.tile([NBD, FU], F32, tag="S2")
        POOL_TILES = (2, 3, 4, 5, 6, 7, 8)  # TA on GpSimd for these n-tiles
        for nt in range(NT):
            u2_sl = u2_all[:, nt * FU:(nt + 1) * FU]
            a_sl = e_all[:, nt * NBD:(nt + 1) * NBD]  # staging (overwritten by exp)
            ta = tapool.tile([128, FU], F32, tag="ta")
            if nt in POOL_TILES:
                nc.gpsimd.tensor_tensor(ta[:], u2_sl, s_sb[:], OP.mult)
            else:
                nc.vector.tensor_tensor(ta[:], u2_sl, s_sb[:], OP.mult)
            nc.vector.tensor_reduce(
                a_sl,
                ta[:].rearrange("p (g j) -> p g j", g=NBD, j=DD),
                AX.X,
                OP.add,
            )
            # E = exp(A / sqrt(dp))
            nc.scalar.activation(a_sl, a_sl, ACTF.Exp, scale=INV_SQRT_DP)
            # z[(b)] = sum_d E ; zr = 1/z
            z_sl = z_all[:, nt * BPC:(nt + 1) * BPC]
            zr_sl = zr_all[:, nt * BPC:(nt + 1) * BPC]
            nc.vector.tensor_reduce(
                z_sl,
                a_sl.rearrange("p (b d) -> p b d", b=BPC, d=D),
                AX.X,
                OP.add,
            )
            nc.vector.reciprocal(zr_sl, z_sl)
            # cb = E * zr + B_prior, written directly as bf16 for the matmul
            cbbf_sl = cbbf_all[:, nt * NBD:(nt + 1) * NBD]
            for b in range(BPC):
                nc.vector.scalar_tensor_tensor(
                    cbbf_sl[:, b * D:(b + 1) * D],
                    a_sl[:, b * D:(b + 1) * D],
                    zr_sl[:, b: b + 1],
                    w_tiles[nt][:, FW + BPC * DP: FW + BPC * DP + D],
                    OP.mult,
                    OP.add,
                )
            # S2 += cb.T @ U2 (bf16 operands, fp32 PSUM accumulate)
            nc.tensor.matmul(
                S2_ps[:],
                cbbf_sl,
                u2bf_all[:, nt * FU:(nt + 1) * FU],
                start=(nt == 0),
                stop=(nt == NT - 1),
            )

        # ---- phase 3: extract diagonal (b,d)=(b',d') via iota mask ----
        iota_t = persist.tile([NBD, FU], I32, tag="iota")
        nc.gpsimd.iota(
            iota_t[:], pattern=[[1, NBD], [0, DD]], base=0, channel_multiplier=-1
        )
        mask_t = persist.tile([NBD, FU], F32, tag="mask")
        nc.vector.tensor_scalar(mask_t[:], iota_t[:], 0, None, OP.is_equal)

        sm_t = smpool.tile([NBD, FU], F32, tag="sm")
        nc.vector.tensor_tensor(sm_t[:], S2_ps[:], mask_t[:], OP.mult)
        s_diag = persist.tile([NBD, DD], F32, tag="sdiag")
        nc.vector.tensor_reduce(
            s_diag[:],
            sm_t[:].rearrange("p (g j) -> p j g", g=NBD, j=DD),
            AX.X,
            OP.add,
        )

        # ---- phase 4: squash ----
        ss_t = persist.tile([NBD, DD], F32, tag="ss")
        nrm2 = persist.tile([NBD, 1], F32, tag="nrm2")
        nc.vector.tensor_tensor(ss_t[:], s_diag[:], s_diag[:], OP.mult)
        nc.vector.tensor_reduce(nrm2[:], ss_t[:], AX.X, OP.add)
        # norm via DVE Newton sqrt (bit-hack seed + 2 iterations) - keeps the
        # Exp ACT table resident (no sqrt/exp table reload in the tail)
        # norm via one Halley iteration from the bit-hack seed (cubic:
        # 3.5e-2 seed error -> ~4e-5), all on DVE
        nrm = persist.tile([NBD, 1], F32, tag="nrm")
        seed_i = persist.tile([NBD, 1], I32, tag="seedi")
        nc.vector.tensor_scalar(
            seed_i[:], nrm2[:].bitcast(I32), 1, None, OP.logical_shift_right
        )
        nc.vector.tensor_scalar(seed_i[:], seed_i[:], 0x1FBD1DF5, None, OP.add)
        seed_f = seed_i[:].bitcast(F32)
        y2 = persist.tile([NBD, 1], F32, tag="y2")
        nc.vector.tensor_tensor(y2[:], seed_f, seed_f, OP.mult)
        hnum = persist.tile([NBD, 1], F32, tag="hnum")
        nc.vector.scalar_tensor_tensor(hnum[:], nrm2[:], 3.0, y2[:], OP.mult, OP.add)
        hden = persist.tile([NBD, 1], F32, tag="hden")
        nc.vector.scalar_tensor_tensor(hden[:], y2[:], 3.0, nrm2[:], OP.mult, OP.add)
        nwr = persist.tile([NBD, 1], F32, tag="nwr")
        nc.vector.reciprocal(nwr[:], hden[:])
        nwt = persist.tile([NBD, 1], F32, tag="nwt")
        nc.vector.tensor_tensor(nwt[:], hnum[:], nwr[:], OP.mult)
        nc.vector.tensor_tensor(nrm[:], seed_f, nwt[:], OP.mult)
        # coef = 1 - 1/(e^r + eps) ~= 1 - e^-r  (abs diff <= eps*e^-2r <= 1e-7)
        en = persist.tile([NBD, 1], F32, tag="en")
        nc.scalar.activation(en[:], nrm[:], ACTF.Exp, scale=-1.0)
        coef = persist.tile([NBD, 1], F32, tag="coef")
        nc.vector.tensor_scalar(coef[:], en[:], -1.0, 1.0, OP.mult, OP.add)
        # norm >= O(1) here: the reference's +1e-7 on the divisor is below
        # fp32 resolution of the result - divide by nrm directly
        r2 = persist.tile([NBD, 1], F32, tag="r2")
        nc.vector.reciprocal(r2[:], nrm[:])
        fac = persist.tile([NBD, 1], F32, tag="fac")
        nc.vector.tensor_tensor(fac[:], coef[:], r2[:], OP.mult)

        res_t = persist.tile([NBD, DD], F32, tag="res")
        nc.vector.tensor_scalar(res_t[:], s_diag[:], fac[:], None, OP.mult)

        nc.sync.dma_start(out_ap.rearrange("b d j -> (b d) j"), res_t[:])


_CACHE: dict = {}


def _get_nc():
    if "nc" not in _CACHE:
        nc = bacc.Bacc(
            "TRN2", target_bir_lowering=False, debug=False, num_devices=NCORES
        )
        # host-pre-arranged: W, u and B_prior fused per tile so each tile is
        # ONE fully contiguous DMA (cols 0:1280 = W, 1280:1296 = u, 1296:1306 = bp)
        WUB = nc.dram_tensor(
            "wub_arr", [NT, 128, FW + BPC * DP + D], F32, kind="ExternalInput"
        ).ap()
        out = nc.dram_tensor("out", [BPC, D, DD], F32, kind="ExternalOutput").ap()
        with tile.TileContext(nc) as tc:
            _build_kernel(tc, out, WUB)
        nc.compile()
        _CACHE["nc"] = nc
    return _CACHE["nc"]


def _arrange(primary_caps, W, B_prior, core):
    """Host-side pre-arrangement into the exact SBUF tile layouts so every
    device DMA reads fully contiguous memory."""
    W = np.asarray(W, dtype=np.float32)
    Bp = np.asarray(B_prior, dtype=np.float32)
    pc = np.asarray(primary_caps, dtype=np.float32)
    w_arr = W.transpose(1, 0, 2, 3).reshape(NT, 128, FW)
    u_arr = (
        pc[core * BPC:(core + 1) * BPC]
        .transpose(1, 0, 2)
        .reshape(NT, 128, BPC * DP)
    )
    bp_arr = Bp[:, 0, :].T.reshape(NT, 128, D)
    return {
        "wub_arr": np.ascontiguousarray(
            np.concatenate([w_arr, u_arr, bp_arr], axis=2)
        )
    }


def _run(primary_caps, W, B_prior, trace=False, **kw):
    nc = _get_nc()
    in_maps = [
        _arrange(primary_caps, W, B_prior, c) for c in range(NCORES)
    ]
    res = run_bass_kernel_spmd(nc, in_maps, list(range(NCORES)), trace=trace, **kw)
    out = np.concatenate([res.results[c]["out"] for c in range(NCORES)], axis=0)
    return out.astype(np.float32), res


def kernel(primary_caps, W, B_prior):
    out, _ = _run(primary_caps, W, B_prior, trace=False)
    return out



# revision 5
# speedup vs baseline: 1.3641x; 1.3641x over previous
"""DigitCaps kernel for 8 Trainium2 NeuronCores.

Math (per batch b):
    U_hat[b,d,n,j] = sum_i W[d,n,j,i] * u[b,n,i]
    A_sum[b,d,m]   = s[b,d,:] . U_hat[b,d,m,:] / sqrt(dp),  s = sum_n U_hat
    C              = softmax_d(A_sum)
    S[b,d,j]       = sum_m (B_prior[d,m] + C[b,d,m]) * U_hat[b,d,m,j]
    out            = squash(S)

Sharding: data-parallel over batch, 2 batches per core, W/B_prior replicated.

Key layout: W is host-cast to bf16 and arranged as
    [partition p=(nhat 16, i 8), free (blk 72, d 10, j 16)],  n = blk*16+nhat
so the votes become per-blk PE matmuls contracting (nhat,i):
    lhsT = u_bd[:, blk]  [128, (nhat',b)=32]   block-diagonal u (stationary)
    rhs  = W_blk         [128, (d,j)=160]      (moving)
    out  = [32, 160] PSUM quadrant -> U_hat[(q,nhat,b), (d,j)]
u_bd is built in ONE DVE masked multiply from a dense u shipped inside the
same DMA stream.  Everything downstream (s, A_sum, softmax over d, S2,
squash) operates on the [(q,nhat,b) partitions, (tile,d,j) free] layout:
 - s via accumulating selector matmuls on PE (2 out cols)
 - A_sum as one batched bf16 DVE multiply + 4 tree adds (reduce over j)
 - softmax over d batched across all tiles in single instructions
 - S2 as 18 accumulating matmuls with block-diag (C+B_prior) stationary
 - diagonal extract via iota mask; squash with Newton sqrt on DVE
"""

import math
import numpy as np

import concourse.bacc as bacc
import concourse.bass as bass
import concourse.tile as tile
from concourse import mybir
from concourse.bass_utils import run_bass_kernel_spmd

F32 = mybir.dt.float32
BF16 = mybir.dt.bfloat16
I32 = mybir.dt.int32
AX = mybir.AxisListType
OP = mybir.AluOpType
ACTF = mybir.ActivationFunctionType

B, N, DP = 16, 1152, 8
D, DD = 10, 16
NCORES = 8
BPC = B // NCORES            # 2 batches per core
NBLK = N // 16               # 72 blocks of 16 n
NPT = 6                      # votes PSUM tiles, 12 blks each
NET = 18                     # evac col-groups (T 6, g 3), 160 cols each
FDJ = D * DD                 # 160
NBD = BPC * D                # 20
UCOLS = NBLK * BPC           # 144 dense-u cols
BPCOLS = NET * D             # 180 B_prior cols
WCOLS = NBLK * FDJ           # 11520 W cols
NCOLS = UCOLS + BPCOLS + WCOLS
W0 = UCOLS + BPCOLS          # W col offset
EPS = 1e-7
INV_SQRT_DP = 1.0 / math.sqrt(DP)


def _build_kernel(tc: "tile.TileContext", out_ap, wub):
    nc = tc.nc
    with (
        tc.tile_pool(name="persist", bufs=1) as persist,
        tc.tile_pool(name="vpool", bufs=5, space="PSUM") as vpool,
        tc.tile_pool(name="mpool", bufs=1, space="PSUM") as mpool,
    ):
        w_sb = persist.tile([128, NCOLS], BF16, tag="wsb")

        # ---- DMA program: u+bp chunk, then 9 W chunks of 8 blks ----
        nc.sync.dma_start(w_sb[:, 0:W0], wub[:, 0:W0])
        WCH = 8 * FDJ  # 1280 cols
        for c in range(9):
            lo = W0 + c * WCH
            nc.sync.dma_start(w_sb[:, lo:lo + WCH], wub[:, lo:lo + WCH])

        # ---- constants / masks (overlap the DMAs) ----
        # preload the Exp ACT table while ACT is idle
        warm_t = persist.tile([1, 1], F32, tag="warm")
        nc.vector.memset(warm_t[:], 0.0)
        nc.scalar.activation(warm_t[:], warm_t[:], ACTF.Exp)

        ip = persist.tile([128, 1], I32, tag="ip")
        nc.gpsimd.iota(ip[:], pattern=[[0, 1]], base=0, channel_multiplier=1)
        # mask2[p,(nh',b)] = (p>>3 == nh')  [u_bd builder; p=(nhat,i) here]
        pn3 = persist.tile([128, 1], I32, tag="pn3")
        nc.vector.tensor_scalar(pn3[:], ip[:], 3, None, OP.logical_shift_right)
        inb = persist.tile([128, 32], I32, tag="inb")
        nc.gpsimd.iota(inb[:], pattern=[[1, 16], [0, 2]], base=0, channel_multiplier=0)
        mask2 = persist.tile([128, 32], BF16, tag="mask2")
        nc.vector.tensor_tensor(
            mask2[:].rearrange("p (n b) -> p n b", n=16, b=2),
            inb[:].rearrange("p (n b) -> p n b", n=16, b=2),
            pn3[:].unsqueeze(2).broadcast_to([128, 16, 2]),
            OP.is_equal,
        )
        # selb[p,b'] = (p%2 == b')  [p=(q,nhat,b) downstream]
        pb = persist.tile([128, 1], I32, tag="pb")
        nc.vector.tensor_scalar(pb[:], ip[:], 1, None, OP.bitwise_and)
        ib2 = persist.tile([128, 2], I32, tag="ib2")
        nc.gpsimd.iota(ib2[:], pattern=[[1, 2]], base=0, channel_multiplier=0)
        selb = persist.tile([128, 2], BF16, tag="selb")
        nc.vector.tensor_tensor(
            selb[:], ib2[:],
            pb[:].broadcast_to([128, 2]), OP.is_equal,
        )
        # bsel[p,(b',d')] = (p%2 == b')
        ibd = persist.tile([128, 20], I32, tag="ibd")
        nc.gpsimd.iota(ibd[:], pattern=[[1, 2], [0, 10]], base=0, channel_multiplier=0)
        bsel = persist.tile([128, 20], BF16, tag="bsel")
        nc.vector.tensor_tensor(
            bsel[:].rearrange("p (b d) -> p b d", b=2, d=10),
            ibd[:].rearrange("p (b d) -> p b d", b=2, d=10),
            pb[:].unsqueeze(2).broadcast_to([128, 2, 10]),
            OP.is_equal,
        )
        # sel2[p',c] = (c%2 == p')   [2,128] for the s broadcast matmul
        ic2 = persist.tile([2, 128], I32, tag="ic2")
        nc.gpsimd.iota(ic2[:], pattern=[[0, 64], [1, 2]], base=0, channel_multiplier=0)
        ip2 = persist.tile([2, 1], I32, tag="ip2")
        nc.gpsimd.iota(ip2[:], pattern=[[0, 1]], base=0, channel_multiplier=1)
        sel2 = persist.tile([2, 128], BF16, tag="sel2")
        nc.vector.tensor_tensor(
            sel2[:], ic2[:],
            ip2[:].broadcast_to([2, 128]), OP.is_equal,
        )
        # dmask[(b',d'),(d,j)] = (d == d')
        ipc = persist.tile([NBD, 1], I32, tag="ipc")
        nc.gpsimd.iota(ipc[:], pattern=[[0, 1]], base=0, channel_multiplier=1)
        ge10 = persist.tile([NBD, 1], I32, tag="ge10")
        nc.vector.tensor_scalar(ge10[:], ipc[:], 10, None, OP.is_ge)
        g10 = persist.tile([NBD, 1], I32, tag="g10")
        nc.vector.tensor_scalar(g10[:], ge10[:], 10, None, OP.mult)
        pd = persist.tile([NBD, 1], I32, tag="pd")
        nc.vector.tensor_tensor(pd[:], ipc[:], g10[:], OP.subtract)
        idj = persist.tile([NBD, FDJ], I32, tag="idj")
        nc.gpsimd.iota(idj[:], pattern=[[1, 10], [0, 16]], base=0, channel_multiplier=0)
        dmask = persist.tile([NBD, FDJ], F32, tag="dmask")
        nc.vector.tensor_tensor(
            dmask[:].rearrange("p (d j) -> p d j", d=10, j=16),
            idj[:].rearrange("p (d j) -> p d j", d=10, j=16),
            pd[:].unsqueeze(2).broadcast_to([NBD, 10, 16]),
            OP.is_equal,
        )

        # ---- u_bd: one masked DVE multiply ----
        # u_bd[p=(nh,i), (blk, nh', b)] = u_dense[p,(blk,b)] * mask2[p,(nh',b)]
        u_bd = persist.tile([128, NBLK * 32], BF16, tag="ubd")
        nc.vector.tensor_tensor(
            u_bd[:].rearrange("p (k n b) -> p k n b", k=NBLK, n=16, b=2),
            w_sb[:, 0:UCOLS]
            .rearrange("p (k b) -> p k b", k=NBLK, b=2)
            .unsqueeze(2)
            .broadcast_to([128, NBLK, 16, 2]),
            mask2[:]
            .rearrange("p (n b) -> p n b", n=16, b=2)
            .unsqueeze(1)
            .broadcast_to([128, NBLK, 16, 2]),
            OP.mult,
        )

        # ---- votes + evac + s accumulation, pipelined per PSUM tile ----
        u2bf = persist.tile([128, NET * FDJ], BF16, tag="u2bf")
        s_ps = mpool.tile([2, FDJ], F32, tag="sps")
        for T in range(NPT):
            vt = vpool.tile([128, 3 * FDJ], F32, tag="vt")
            for g in range(3):
                for q in range(4):
                    blk = 12 * T + 4 * g + q
                    nc.tensor.matmul(
                        vt[q * 32:(q + 1) * 32, g * FDJ:(g + 1) * FDJ],
                        u_bd[:, blk * 32:(blk + 1) * 32],
                        w_sb[:, W0 + blk * FDJ:W0 + (blk + 1) * FDJ],
                        start=True,
                        stop=True,
                        tile_position=(0, q * 32),
                    )
            # evac fp32 PSUM -> bf16 SBUF on ACT
            nc.scalar.copy(u2bf[:, T * 3 * FDJ:(T + 1) * 3 * FDJ], vt[:])
            # s[b,(d,j)] += sum_p selb[p,b] * u2bf[p,(d,j)]
            for g in range(3):
                et = 3 * T + g
                nc.tensor.matmul(
                    s_ps[:],
                    selb[:],
                    u2bf[:, et * FDJ:(et + 1) * FDJ],
                    start=(et == 0),
                    stop=(et == NET - 1),
                )

        # ---- s broadcast to all 128 partitions ----
        s_sb = persist.tile([2, FDJ], BF16, tag="ssb")
        nc.vector.tensor_copy(s_sb[:], s_ps[:])
        sbc_ps = mpool.tile([128, FDJ], F32, tag="sbc")
        nc.tensor.matmul(sbc_ps[:], sel2[:], s_sb[:], start=True, stop=True)
        sbc_sb = persist.tile([128, FDJ], BF16, tag="sbcsb")
        nc.vector.tensor_copy(sbc_sb[:], sbc_ps[:])

        # ---- A_sum: one batched multiply + tree reduce over j ----
        aprod = persist.tile([128, NET * FDJ], BF16, tag="aprod")
        nc.vector.tensor_tensor(
            aprod[:].rearrange("p (t f) -> p t f", t=NET, f=FDJ),
            u2bf[:].rearrange("p (t f) -> p t f", t=NET, f=FDJ),
            sbc_sb[:].unsqueeze(1).broadcast_to([128, NET, FDJ]),
            OP.mult,
        )
        a1 = persist.tile([128, NET * D * 8], BF16, tag="a1")
        ap_v = aprod[:].rearrange("p (g j) -> p g j", g=NET * D, j=16)
        a1_v = a1[:].rearrange("p (g j) -> p g j", g=NET * D, j=8)
        nc.vector.tensor_tensor(a1_v, ap_v[:, :, 0:8], ap_v[:, :, 8:16], OP.add)
        a2 = persist.tile([128, NET * D * 4], BF16, tag="a2")
        a2_v = a2[:].rearrange("p (g j) -> p g j", g=NET * D, j=4)
        nc.vector.tensor_tensor(a2_v, a1_v[:, :, 0:4], a1_v[:, :, 4:8], OP.add)
        a3 = persist.tile([128, NET * D * 2], BF16, tag="a3")
        a3_v = a3[:].rearrange("p (g j) -> p g j", g=NET * D, j=2)
        nc.vector.tensor_tensor(a3_v, a2_v[:, :, 0:2], a2_v[:, :, 2:4], OP.add)
        a_all = persist.tile([128, NET * D], F32, tag="aall")
        nc.vector.tensor_tensor(
            a_all[:].unsqueeze(2), a3_v[:, :, 0:1], a3_v[:, :, 1:2], OP.add
        )

        # ---- softmax over d (batched across all 18 tiles) ----
        e_all = persist.tile([128, NET * D], BF16, tag="eall")
        nc.scalar.activation(e_all[:], a_all[:], ACTF.Exp, scale=INV_SQRT_DP)
        z = persist.tile([128, NET], F32, tag="z")
        nc.vector.tensor_reduce(
            z[:], e_all[:].rearrange("p (t d) -> p t d", t=NET, d=D), AX.X, OP.add
        )
        zr = persist.tile([128, NET], F32, tag="zr")
        nc.vector.reciprocal(zr[:], z[:])
        cb1 = persist.tile([128, NET * D], BF16, tag="cb1")
        nc.vector.tensor_tensor(
            cb1[:].rearrange("p (t d) -> p t d", t=NET, d=D),
            e_all[:].rearrange("p (t d) -> p t d", t=NET, d=D),
            zr[:].unsqueeze(2).broadcast_to([128, NET, D]),
            OP.mult,
        )
        cb_all = persist.tile([128, NET * D], BF16, tag="cball")
        nc.vector.tensor_tensor(
            cb_all[:], cb1[:], w_sb[:, UCOLS:UCOLS + BPCOLS], OP.add
        )
        # block-diag over b: cbbf[p,(t,b',d')] = cb_all[p,(t,d')] * (p%2==b')
        cbbf = persist.tile([128, NET * NBD], BF16, tag="cbbf")
        nc.vector.tensor_tensor(
            cbbf[:].rearrange("p (t b d) -> p t b d", t=NET, b=2, d=10),
            cb_all[:]
            .rearrange("p (t d) -> p t d", t=NET, d=10)
            .unsqueeze(2)
            .broadcast_to([128, NET, 2, 10]),
            bsel[:]
            .rearrange("p (b d) -> p b d", b=2, d=10)
            .unsqueeze(1)
            .broadcast_to([128, NET, 2, 10]),
            OP.mult,
        )

        # ---- S2: accumulate over all m ----
        S2_ps = mpool.tile([NBD, FDJ], F32, tag="S2")
        for t in range(NET):
            nc.tensor.matmul(
                S2_ps[:],
                cbbf[:, t * NBD:(t + 1) * NBD],
                u2bf[:, t * FDJ:(t + 1) * FDJ],
                start=(t == 0),
                stop=(t == NET - 1),
            )

        # ---- diagonal extract d==d' ----
        sm_t = persist.tile([NBD, FDJ], F32, tag="sm")
        nc.vector.tensor_tensor(sm_t[:], S2_ps[:], dmask[:], OP.mult)
        s_diag = persist.tile([NBD, DD], F32, tag="sdiag")
        nc.vector.tensor_reduce(
            s_diag[:],
            sm_t[:].rearrange("p (d j) -> p j d", d=10, j=16),
            AX.X,
            OP.add,
        )

        # ---- squash ----
        ss_t = persist.tile([NBD, DD], F32, tag="ss")
        nrm2 = persist.tile([NBD, 1], F32, tag="nrm2")
        nc.vector.tensor_tensor(ss_t[:], s_diag[:], s_diag[:], OP.mult)
        nc.vector.tensor_reduce(nrm2[:], ss_t[:], AX.X, OP.add)
        # norm via one Halley iteration from the bit-hack seed (all on DVE;
        # keeps the Exp ACT table resident)
        nrm = persist.tile([NBD, 1], F32, tag="nrm")
        seed_i = persist.tile([NBD, 1], I32, tag="seedi")
        nc.vector.tensor_scalar(
            seed_i[:], nrm2[:].bitcast(I32), 1, None, OP.logical_shift_right
        )
        nc.vector.tensor_scalar(seed_i[:], seed_i[:], 0x1FBD1DF5, None, OP.add)
        seed_f = seed_i[:].bitcast(F32)
        y2 = persist.tile([NBD, 1], F32, tag="y2")
        nc.vector.tensor_tensor(y2[:], seed_f, seed_f, OP.mult)
        hnum = persist.tile([NBD, 1], F32, tag="hnum")
        nc.vector.scalar_tensor_tensor(hnum[:], nrm2[:], 3.0, y2[:], OP.mult, OP.add)
        hden = persist.tile([NBD, 1], F32, tag="hden")
        nc.vector.scalar_tensor_tensor(hden[:], y2[:], 3.0, nrm2[:], OP.mult, OP.add)
        nwr = persist.tile([NBD, 1], F32, tag="nwr")
        nc.vector.reciprocal(nwr[:], hden[:])
        nwt = persist.tile([NBD, 1], F32, tag="nwt")
        nc.vector.tensor_tensor(nwt[:], hnum[:], nwr[:], OP.mult)
        nc.vector.tensor_tensor(nrm[:], seed_f, nwt[:], OP.mult)
        # coef = 1 - 1/(e^r + eps) ~= 1 - e^-r
        en = persist.tile([NBD, 1], F32, tag="en")
        nc.scalar.activation(en[:], nrm[:], ACTF.Exp, scale=-1.0)
        coef = persist.tile([NBD, 1], F32, tag="coef")
        nc.vector.tensor_scalar(coef[:], en[:], -1.0, 1.0, OP.mult, OP.add)
        r2 = persist.tile([NBD, 1], F32, tag="r2")
        nc.vector.reciprocal(r2[:], nrm[:])
        fac = persist.tile([NBD, 1], F32, tag="fac")
        nc.vector.tensor_tensor(fac[:], coef[:], r2[:], OP.mult)
        res_t = persist.tile([NBD, DD], F32, tag="res")
        nc.vector.tensor_scalar(res_t[:], s_diag[:], fac[:], None, OP.mult)

        nc.sync.dma_start(out_ap.rearrange("b d j -> (b d) j"), res_t[:])


_CACHE: dict = {}


def _get_nc():
    if "nc" not in _CACHE:
        nc = bacc.Bacc(
            "TRN2", target_bir_lowering=False, debug=False, num_devices=NCORES
        )
        wub = nc.dram_tensor("wub", [128, NCOLS], BF16, kind="ExternalInput").ap()
        out = nc.dram_tensor("out", [BPC, D, DD], F32, kind="ExternalOutput").ap()
        with tile.TileContext(nc) as tc:
            _build_kernel(tc, out, wub)
        nc.compile()
        _CACHE["nc"] = nc
    return _CACHE["nc"]


def _arrange(primary_caps, W, B_prior, core):
    """Host-side pre-arrangement: bf16 cast + the exact SBUF layouts."""
    import ml_dtypes

    W = np.asarray(W, dtype=np.float32)
    Bp = np.asarray(B_prior, dtype=np.float32)
    pc = np.asarray(primary_caps, dtype=np.float32)

    # u_dense: [p=(nhat,i), (blk,b)]
    u_arr = (
        pc[core * BPC:(core + 1) * BPC]
        .reshape(BPC, NBLK, 16, DP)
        .transpose(2, 3, 1, 0)
        .reshape(128, UCOLS)
    )
    # bp: [p=(q,nhat,b), (et,d)] with blk = 12*(et//3)+4*(et%3)+q
    pidx = np.arange(128)
    qv = pidx // 32
    nhat = (pidx // 2) % 16
    et = np.arange(NET)
    blk_et = 12 * (et // 3) + 4 * (et % 3)
    n_idx = (blk_et[None, :] + qv[:, None]) * 16 + nhat[:, None]  # [128, 18]
    bp_arr = (
        Bp[:, 0, :][:, n_idx]          # [10, 128, 18]
        .transpose(1, 2, 0)
        .reshape(128, BPCOLS)
    )
    # W: [p=(nhat,i), (blk, d, j)]
    w_arr = (
        W.reshape(D, NBLK, 16, DD, DP)
        .transpose(2, 4, 1, 0, 3)
        .reshape(128, WCOLS)
    )
    full = np.concatenate([u_arr, bp_arr, w_arr], axis=1)
    return {"wub": np.ascontiguousarray(full).astype(ml_dtypes.bfloat16)}


def _run(primary_caps, W, B_prior, trace=False, **kw):
    nc = _get_nc()
    in_maps = [
        _arrange(primary_caps, W, B_prior, c) for c in range(NCORES)
    ]
    res = run_bass_kernel_spmd(nc, in_maps, list(range(NCORES)), trace=trace, **kw)
    out = np.concatenate([res.results[c]["out"] for c in range(NCORES)], axis=0)
    return out.astype(np.float32), res


def kernel(primary_caps, W, B_prior):
    out, _ = _run(primary_caps, W, B_prior, trace=False)
    return out
